# revision 17
# baseline (speedup 1.0000x reference)
"""ChebNet (K=2, H=16) forward on 8 Trainium2 NeuronCores.

Strategy: nodes+edges sharded by destination row across 8 cores.  Both
ChebConv layers reduce to a scalar gather/scatter over the edge list:

    s[i] = sum_{e: row[e]=i} table[col[e]]       (table = dinv*x, then dinv*g2)

Per core the edge list is bucketed by source column-block J (128 nodes),
padded to S chunks of 128 edges per bucket.  Per chunk (128 edges):
  - DVE builds a node-major scaled one-hot Oq[f,e] = (q_bcast==iotaP)*y[f]
    in ONE wide bf16 tensor_scalar per J-block (4x DVE mode)
  - PE gather matmul: msg[e] = Oq_chunk^T @ ones  (PSUM column)
  - Act engine copies msg PSUM->SBUF once per J-block
  - DVE builds Ol[e,lo] = (iota==lo_e)*msg_e and Ohm[e,hi] = (iota==hi_e)
    as per-chunk bf16 tensor_scalars (4x DVE mode)
  - PE scatter matmul accumulates G[hi,lo] += Ohm^T @ Ol in PSUM
Tiny per-node channel math + one AllGather (z table) between the passes.
"""

import json
import os

import numpy as np
import ml_dtypes

N = 100000
NC = 8
NLOC = N // NC           # 12500 nodes per core
HIB = 98                 # dest hi blocks  (local = hi*128 + lo)
Q = 128                  # source col-block width
NJ = (N + Q - 1) // Q    # 782
GJ = 98                  # J blocks per hw-loop iteration (784 = 98*8)
NJP = ((NJ + GJ - 1) // GJ) * GJ   # 784
NG = NJP // GJ           # 8 loop iterations
JB = 14                  # J-blocks per qb staging DMA (98 = 7*14; <=12
                         # dynamic-broadcast DMAs per For_i body are allowed)
H = 16

_TRACE = bool(int(os.environ.get("KERNEL_TRACE", "0")))
_LAST_TRACE = {}


def _host_prep(x, edge_index, W1, b1, W2, b2):
    x = np.asarray(x, np.float32).reshape(-1)
    ei = np.asarray(edge_index)
    row = ei[0].astype(np.int64)
    col = ei[1].astype(np.int64)

    deg = np.bincount(row, minlength=N).astype(np.float32)

    core = row // NLOC
    J = col // Q
    q = (col % Q).astype(np.float32)
    loc = row % NLOC
    lo = (loc % 128).astype(np.float32)
    hi = (loc // 128).astype(np.float32)

    # per (core, J) bucket sizes -> global uniform S
    cnt = np.bincount(core * NJP + J, minlength=NC * NJP).reshape(NC, NJP)
    S = int(np.max((cnt + 127) // 128))
    C = NJP * S

    order = np.lexsort((J, core))
    J_s = J[order]
    core_s = core[order]
    bucket_id = core_s * NJP + J_s
    starts = np.zeros(NC * NJP + 1, np.int64)
    np.cumsum(np.bincount(bucket_id, minlength=NC * NJP), out=starts[1:])
    k = np.arange(order.size, dtype=np.int64) - starts[bucket_id]

    # slot layout: qd flat index = (J*S + s)*128 + e ; lo/hi at [e, J*S + s]
    chunk = J_s * S + k // 128
    part = k % 128

    q_o = q[order]
    lo_o = lo[order]
    hi_o = hi[order]

    CG = GJ * S
    per_core = []
    for c in range(NC):
        m = core_s == c
        qs = np.zeros(C * 128, np.float32)
        los = np.full(128 * C, 127.0, np.float32)
        his = np.full(128 * C, 97.0, np.float32)
        ch = chunk[m]
        pt = part[m]
        qs[ch * 128 + pt] = q_o[m]
        los[pt * C + ch] = lo_o[m]
        his[pt * C + ch] = hi_o[m]
        # merged lo/hi stream, group-major: [128, NG, {lo,hi}, CG]
        lh = np.empty((128, NG, 2, CG), np.float32)
        lh[:, :, 0, :] = los.reshape(128, NG, CG)
        lh[:, :, 1, :] = his.reshape(128, NG, CG)

        deg_loc = np.zeros(HIB * 128, np.float32)
        x_loc = np.zeros(HIB * 128, np.float32)
        deg_loc[:NLOC] = deg[c * NLOC:(c + 1) * NLOC]
        x_loc[:NLOC] = x[c * NLOC:(c + 1) * NLOC]

        per_core.append(dict(
            qd=qs.astype(np.float16).reshape(1, C * 128),
            lhd=lh.reshape(128, 2 * C),
            deg_loc=deg_loc.reshape(HIB, 128),
            x_loc=x_loc.reshape(HIB, 128),
        ))

    # col-block global tables (node n -> [n%128, n//128]), zero padded
    x_cb = np.zeros(128 * NJP, np.float32)
    deg_cb = np.zeros(128 * NJP, np.float32)
    idx = np.arange(N)
    cbf = (idx % 128) * NJP + idx // 128
    x_cb[cbf] = x
    deg_cb[cbf] = deg
    x_cb = x_cb.reshape(128, NJP)
    deg_cb = deg_cb.reshape(128, NJP)

    params = np.zeros(81, np.float32)
    params[0:16] = np.asarray(W1, np.float32)[0, 0]
    params[16:32] = np.asarray(W1, np.float32)[1, 0]
    params[32:48] = np.asarray(b1, np.float32)
    params[48:64] = np.asarray(W2, np.float32)[0, :, 0]
    params[64:80] = np.asarray(W2, np.float32)[1, :, 0]
    params[80] = np.asarray(b2, np.float32).reshape(-1)[0]
    params = params.reshape(1, 81)

    in_maps = []
    for c in range(NC):
        d = per_core[c]
        in_maps.append({
            "qd": d["qd"], "lhd": d["lhd"],
            "x_cb": x_cb, "deg_cb": deg_cb,
            "x_loc": d["x_loc"], "deg_loc": d["deg_loc"],
            "params": params,
        })
    return S, C, in_maps


def _split_drain_waits(js: bytes) -> bytes:
    """This walrus build rejects >1 sync-wait per instruction; carry excess
    waits on preceding same-engine NoOps (engines dispatch in order)."""
    m = json.loads(js)

    def fix_block(bb):
        insts = bb.get("instructions")
        if not insts:
            return
        out = []
        for inst in insts:
            si = inst.get("sync_info") or {}
            waits = si.get("on_wait") or []
            if len(waits) > 1:
                for kk, w in enumerate(waits[:-1]):
                    carrier = {
                        "opcode": "NoOp",
                        "engine": inst.get("engine", "SP"),
                        "name": f"{inst['name']}_sw{kk}",
                        "ins": [],
                        "outs": [],
                        "sync_info": {"on_update": [], "on_wait": [w]},
                    }
                    if "debug" in inst:
                        carrier["debug"] = inst["debug"]
                    out.append(carrier)
                inst["sync_info"]["on_wait"] = [waits[-1]]
            out.append(inst)
        bb["instructions"] = out

    def walk(o):
        if isinstance(o, dict):
            if isinstance(o.get("instructions"), list):
                fix_block(o)
            for v in o.values():
                walk(v)
        elif isinstance(o, list):
            for v in o:
                walk(v)

    walk(m)
    return json.dumps(m).encode()


def _build_nc(S):
    import concourse.bass as bass
    import concourse.mybir as mybir
    import concourse.tile as tile
    from contextlib import ExitStack

    f32 = mybir.dt.float32
    f16 = mybir.dt.float16
    i32 = mybir.dt.int32
    op = mybir.AluOpType
    C = NJP * S
    CG = GJ * S          # chunk-columns per loop iteration

    nc = bass.Bass()
    qd = nc.declare_dram_parameter("qd", [1, C * 128], f16, isOutput=False)
    lhd = nc.declare_dram_parameter("lhd", [128, 2 * C], f32, isOutput=False)
    x_cb = nc.declare_dram_parameter("x_cb", [128, NJP], f32, isOutput=False)
    deg_cb = nc.declare_dram_parameter("deg_cb", [128, NJP], f32, isOutput=False)
    x_locp = nc.declare_dram_parameter("x_loc", [HIB, 128], f32, isOutput=False)
    deg_locp = nc.declare_dram_parameter("deg_loc", [HIB, 128], f32, isOutput=False)
    paramsp = nc.declare_dram_parameter("params", [1, 81], f32, isOutput=False)
    outp = nc.declare_dram_parameter("outp", [HIB, 128], f32, isOutput=True)

    with ExitStack() as ctx:
        tc = ctx.enter_context(tile.TileContext(nc))
        const = ctx.enter_context(tc.tile_pool(name="const", bufs=1))
        qbp = ctx.enter_context(tc.tile_pool(name="qbp", bufs=3))
        oqp = ctx.enter_context(tc.tile_pool(name="oqp", bufs=4))
        work = ctx.enter_context(tc.tile_pool(name="work", bufs=10))
        lhp = ctx.enter_context(tc.tile_pool(name="lhp", bufs=2))
        msgp = ctx.enter_context(tc.tile_pool(name="msgp", bufs=4))
        nodew = ctx.enter_context(tc.tile_pool(name="nodew", bufs=1))
        psum = ctx.enter_context(tc.tile_pool(name="psum", bufs=1, space="PSUM"))
        psmsg = ctx.enter_context(tc.tile_pool(name="psmsg", bufs=4, space="PSUM"))
        dram = ctx.enter_context(tc.tile_pool(name="dram", bufs=1, space="DRAM"))

        # ---------- constants ----------
        iota_i = const.tile([128, 128], i32)
        nc.gpsimd.iota(iota_i[:], pattern=[[1, 128]], base=0, channel_multiplier=0)
        iotaf = const.tile([128, 128], f16)
        nc.vector.tensor_copy(out=iotaf[:], in_=iota_i[:])
        iotap_i = const.tile([128, 1], i32)
        nc.gpsimd.iota(iotap_i[:], pattern=[[0, 1]], base=0, channel_multiplier=1)
        iotap = const.tile([128, 1], f32)
        nc.vector.tensor_copy(out=iotap[:], in_=iotap_i[:])
        ones_col = const.tile([128, 1], f16)
        nc.gpsimd.memset(ones_col[:], 1.0)
        ZTb = const.tile([128, 128], f16)
        nc.gpsimd.memset(ZTb[:], 0.0)
        P81 = const.tile([128, 81], f32)
        nc.sync.dma_start(out=P81[:], in_=paramsp[0:1, :].to_broadcast([128, 81]))

        # ---------- helpers ----------
        def newton_dinv(dst, deg_tile, F, P=128):
            m = nodew.tile([P, F], f32, name=f"nt_m{F}", tag=f"nt_m{F}")
            r0 = nodew.tile([P, F], f32, name=f"nt_r0{F}", tag=f"nt_r0{F}")
            t = nodew.tile([P, F], f32, name=f"nt_t{F}", tag=f"nt_t{F}")
            nc.vector.tensor_scalar(out=m[:], in0=deg_tile[:], scalar1=1.0,
                                    scalar2=None, op0=op.max)
            nc.scalar.activation(t[:], m[:], mybir.ActivationFunctionType.Sqrt)
            nc.vector.reciprocal(r0[:], t[:])
            nc.vector.tensor_tensor(out=t[:], in0=r0[:], in1=r0[:], op=op.mult)
            nc.vector.tensor_tensor(out=t[:], in0=t[:], in1=m[:], op=op.mult)
            nc.vector.tensor_scalar(out=t[:], in0=t[:], scalar1=-0.5,
                                    scalar2=1.5, op0=op.mult, op1=op.add)
            nc.vector.tensor_tensor(out=t[:], in0=t[:], in1=r0[:], op=op.mult)
            nc.vector.tensor_scalar(out=m[:], in0=deg_tile[:], scalar1=0.0,
                                    scalar2=None, op0=op.not_equal)
            nc.vector.tensor_tensor(out=dst[:], in0=t[:], in1=m[:], op=op.mult)

        # ---------- node tables ----------
        xcb_t = nodew.tile([128, NJP], f32)
        degcb_t = nodew.tile([128, NJP], f32)
        nc.sync.dma_start(out=xcb_t[:], in_=x_cb[:])
        nc.sync.dma_start(out=degcb_t[:], in_=deg_cb[:])
        dinv_cb = nodew.tile([128, NJP], f32)
        newton_dinv(dinv_cb, degcb_t, NJP)
        y_cb = nodew.tile([128, NJP], f32)
        nc.vector.tensor_tensor(out=y_cb[:], in0=dinv_cb[:], in1=xcb_t[:], op=op.mult)

        xl_t = nodew.tile([HIB, 128], f32)
        degl_t = nodew.tile([HIB, 128], f32)
        nc.sync.dma_start(out=xl_t[:], in_=x_locp[:])
        nc.sync.dma_start(out=degl_t[:], in_=deg_locp[:])
        dinv_loc = nodew.tile([HIB, 128], f32)
        newton_dinv(dinv_loc, degl_t, 128, P=HIB)

        # gather tables live in DRAM (dynamic-offset DMA source must be DRAM)
        ycb_d = dram.tile([128, NJP], f32)
        nc.sync.dma_start(out=ycb_d[:], in_=y_cb[:])
        zcb_d = dram.tile([128, NJP], f32)
        z_flat = dram.tile([1, NLOC], f32)
        z_all = dram.tile([1, NJP * 128], f32)

        import concourse.bass as _b

        def emit_pass(table_cb, s_out, pidx):
            G = psum.tile([HIB, 128], f32, tag=f"G{pidx}", name=f"G{pidx}")
            nc.tensor.matmul(out=G[:], lhsT=ZTb[:, :HIB], rhs=ZTb[:],
                             start=True, stop=False, skip_group_check=True)
            with tc.For_i(0, NG) as g:
                lh_t = lhp.tile([128, 2 * CG], f32, name="lh_t", tag="lh_t")
                nc.sync.dma_start(out=lh_t[:], in_=lhd[:, _b.ts(g, 2 * CG)])
                lo_t = lh_t[:, 0:CG]
                hi_t = lh_t[:, CG:2 * CG]
                ycols = lhp.tile([128, GJ], f32, name="ycols", tag="ycols")
                nc.sync.dma_start(out=ycols[:], in_=table_cb[:, _b.ts(g, GJ)])
                ycols16 = lhp.tile([128, GJ], f16, name="ycols16", tag="ycols16")
                nc.vector.tensor_copy(out=ycols16[:], in_=ycols[:])
                # dynamic->static offset staging: one dynamic DRAM->DRAM copy,
                # then static-offset broadcasts (no bounds-check registers)
                qscr = dram.tile([1, CG * 128], f16, name="qscr", tag="qscr",
                                 bufs=2)
                nc.sync.dma_start(out=qscr[0:1, 0:CG * 64],
                                  in_=qd[0:1, _b.ts(2 * g, CG * 64)])
                nc.sync.dma_start(out=qscr[0:1, CG * 64:CG * 128],
                                  in_=qd[0:1, _b.ts(2 * g + 1, CG * 64)])
                for t in range(GJ // JB):
                    qb = qbp.tile([128, JB * S * 128], f16, name="qb", tag="qb")
                    nc.sync.dma_start(
                        out=qb[:],
                        in_=qscr[0:1, t * JB * S * 128:(t + 1) * JB * S * 128]
                        .to_broadcast([128, JB * S * 128]),
                    )
                    for u in range(JB):
                        jj = t * JB + u
                        oqs = oqp.tile([128, S * 128], f16, name="oqs", tag="oqs")
                        nc.vector.tensor_scalar(
                            out=oqs[:], in0=qb[:, u * S * 128:(u + 1) * S * 128],
                            scalar1=iotap[:], scalar2=None, op0=op.is_equal)
                        msg_ps = psmsg.tile([128, S], f32, name="msg_ps", tag="msg_ps")
                        for s in range(S):
                            nc.tensor.matmul(
                                out=msg_ps[:, s:s + 1],
                                lhsT=oqs[:, 128 * s:128 * (s + 1)],
                                rhs=ycols16[:, jj:jj + 1], start=True, stop=True,
                                skip_group_check=True)
                        msg_sb = msgp.tile([128, S], f32, name="msg_sb", tag="msg_sb")
                        nc.scalar.activation(msg_sb[:], msg_ps[:],
                                             mybir.ActivationFunctionType.Copy)
                        for s in range(S):
                            cix = jj * S + s
                            ol = work.tile([128, 128], f16, tag="ol", name="ol")
                            nc.vector.tensor_scalar(
                                out=ol[:], in0=iotaf[:],
                                scalar1=lo_t[:, cix:cix + 1],
                                scalar2=msg_sb[:, s:s + 1],
                                op0=op.is_equal, op1=op.mult)
                            ohm = work.tile([128, HIB], f16, tag="ohm", name="ohm")
                            nc.vector.tensor_scalar(
                                out=ohm[:], in0=iotaf[:, :HIB],
                                scalar1=hi_t[:, cix:cix + 1], scalar2=None,
                                op0=op.is_equal)
                            nc.tensor.matmul(out=G[:], lhsT=ohm[:], rhs=ol[:],
                                             start=False, stop=False,
                                             skip_group_check=True)
            nc.tensor.matmul(out=G[:], lhsT=ZTb[:, :HIB], rhs=ZTb[:],
                             start=False, stop=True, skip_group_check=True)
            nc.vector.tensor_copy(out=s_out[:], in_=G[:])

        # =================== pass 1 ===================
        s1 = nodew.tile([HIB, 128], f32)
        emit_pass(ycb_d, s1, 0)

        # Tx1 = -dinv_loc * s1
        tx1 = nodew.tile([HIB, 128], f32)
        nc.vector.scalar_tensor_tensor(out=tx1[:], in0=s1[:], scalar=-1.0,
                                       in1=dinv_loc[:], op0=op.mult, op1=op.mult)
        g2 = [nodew.tile([HIB, 128], f32, name=f"g2_{i}", tag=f"g2{i}") for i in range(2)]
        p2 = [nodew.tile([HIB, 128], f32, name=f"p2_{i}", tag=f"p2{i}") for i in range(2)]
        nc.gpsimd.memset(g2[0][:], 0.0)
        nc.gpsimd.memset(p2[0][:], 0.0)
        tv = nodew.tile([HIB, 128], f32)
        hch = nodew.tile([HIB, 128], f32)
        for ch in range(H):
            u_c = P81[0:HIB, ch:ch + 1]
            v_c = P81[0:HIB, 16 + ch:17 + ch]
            b1_c = P81[0:HIB, 32 + ch:33 + ch]
            w2a_c = P81[0:HIB, 48 + ch:49 + ch]
            w2b_c = P81[0:HIB, 64 + ch:65 + ch]
            nc.vector.tensor_scalar(out=tv[:], in0=tx1[:], scalar1=v_c,
                                    scalar2=None, op0=op.mult)
            nc.vector.scalar_tensor_tensor(out=hch[:], in0=xl_t[:], scalar=u_c,
                                           in1=tv[:], op0=op.mult, op1=op.add)
            nc.vector.tensor_scalar(out=hch[:], in0=hch[:], scalar1=b1_c,
                                    scalar2=0.0, op0=op.add, op1=op.max)
            a, b = ch % 2, 1 - ch % 2
            nc.vector.scalar_tensor_tensor(out=g2[b][:], in0=hch[:], scalar=w2b_c,
                                           in1=g2[a][:], op0=op.mult, op1=op.add)
            nc.vector.scalar_tensor_tensor(out=p2[b][:], in0=hch[:], scalar=w2a_c,
                                           in1=p2[a][:], op0=op.mult, op1=op.add)
        g2f = g2[H % 2]
        p2f = p2[H % 2]

        # z = dinv_loc * g2  -> z_flat -> allgather -> z_all -> z_cb
        zl = nodew.tile([HIB, 128], f32)
        nc.vector.tensor_tensor(out=zl[:], in0=dinv_loc[:], in1=g2f[:], op=op.mult)
        nc.sync.dma_start(
            out=z_flat[0:1, 0:(HIB - 1) * 128].rearrange("o (h l) -> (o h) l", l=128),
            in_=zl[0:HIB - 1, :],
        )
        nc.sync.dma_start(
            out=z_flat[0:1, (HIB - 1) * 128:NLOC],
            in_=zl[HIB - 1:HIB, 0:NLOC - (HIB - 1) * 128],
        )
        zrow = const.tile([1, 512], f32)
        nc.gpsimd.memset(zrow[:], 0.0)
        nc.sync.dma_start(out=z_all[0:1, N:NJP * 128], in_=zrow[0:1, 0:NJP * 128 - N])
        nc.gpsimd.collective_compute(
            "AllGather", op.bypass,
            replica_groups=[list(range(NC))],
            ins=[z_flat[0:1, :]],
            outs=[z_all[0:1, 0:N]],
        )
        # zcb_d[p, j] = z_all[j*128 + p]  (split: flat side <= 65535 elems/DMA)
        NJH = NJP // 2
        for h in range(2):
            nc.sync.dma_start(
                out=zcb_d[:, h * NJH:(h + 1) * NJH],
                in_=z_all[0:1, h * NJH * 128:(h + 1) * NJH * 128]
                .rearrange("o (j p) -> (o p) j", p=128),
            )

        # =================== pass 2 ===================
        s2 = nodew.tile([HIB, 128], f32)
        emit_pass(zcb_d, s2, 1)

        o1 = nodew.tile([HIB, 128], f32)
        nc.vector.scalar_tensor_tensor(out=o1[:], in0=s2[:], scalar=-1.0,
                                       in1=dinv_loc[:], op0=op.mult, op1=op.mult)
        nc.vector.tensor_tensor(out=o1[:], in0=o1[:], in1=p2f[:], op=op.add)
        nc.vector.tensor_scalar(out=o1[:], in0=o1[:], scalar1=P81[0:HIB, 80:81],
                                scalar2=None, op0=op.add)
        nc.sync.dma_start(out=outp[:], in_=o1[:])

    # patch: split multi-wait Drains for this walrus build
    orig = type(nc).to_json_bytes
    if not getattr(type(nc), "_drain_patched", False):
        def patched(self):
            return _split_drain_waits(orig(self))
        type(nc).to_json_bytes = patched
        type(nc)._drain_patched = True
    return nc


def _install_ntff_hook():
    """Recreate the missing antenv.axon_hooks shim so trace=True works."""
    import sys
    import types
    try:
        import antenv.axon_hooks  # noqa: F401
        return True
    except ImportError:
        pass
    try:
        from trn_agent_boot.trn_boot import _ntff_profile_via_ctypes
        hook = _ntff_profile_via_ctypes("/opt/axon/libaxon_pjrt.so")
        if hook is None:
            return False
        mod = types.ModuleType("antenv.axon_hooks")
        mod._hook = hook
        mod.get_axon_ntff_profile_hook = lambda: mod._hook
        mod.set_axon_ntff_profile_hook = lambda h: setattr(mod, "_hook", h)
        import antenv
        antenv.axon_hooks = mod
        sys.modules["antenv.axon_hooks"] = mod
        return True
    except Exception:
        return False


def kernel(x, edge_index, W1, b1, W2, b2):
    from concourse.bass_utils import run_bass_kernel_spmd

    S, C, in_maps = _host_prep(x, edge_index, W1, b1, W2, b2)
    nc = _build_nc(S)
    trace = _TRACE and _install_ntff_hook()
    res = run_bass_kernel_spmd(nc, in_maps, list(range(NC)), trace=trace)
    global _LAST_TRACE
    _LAST_TRACE = {
        "exec_time_ns": res.exec_time_ns,
        "profile_json": getattr(res, "profile_json", None),
    }
    out = np.concatenate(
        [res.results[c]["outp"].reshape(-1)[:NLOC] for c in range(NC)]
    ).astype(np.float32)
    return out.reshape(N, 1)


# revision 19
# speedup vs baseline: 1.0718x; 1.0718x over previous
"""ChebNet (K=2, H=16) forward on 8 Trainium2 NeuronCores.

Strategy: nodes+edges sharded by destination row across 8 cores.  Both
ChebConv layers reduce to a scalar gather/scatter over the edge list:

    s[i] = sum_{e: row[e]=i} table[col[e]]       (table = dinv*x, then dinv*g2)

Per core the edge list is bucketed by source column-block J (128 nodes),
padded to S chunks of 128 edges per bucket.  Per chunk (128 edges):
  - DVE builds a node-major scaled one-hot Oq[f,e] = (q_bcast==iotaP)*y[f]
    in ONE wide bf16 tensor_scalar per J-block (4x DVE mode)
  - PE gather matmul: msg[e] = Oq_chunk^T @ ones  (PSUM column)
  - Act engine copies msg PSUM->SBUF once per J-block
  - DVE builds Ol[e,lo] = (iota==lo_e)*msg_e and Ohm[e,hi] = (iota==hi_e)
    as per-chunk bf16 tensor_scalars (4x DVE mode)
  - PE scatter matmul accumulates G[hi,lo] += Ohm^T @ Ol in PSUM
Tiny per-node channel math + one AllGather (z table) between the passes.
"""

import json
import os

import numpy as np
import ml_dtypes

N = 100000
NC = 8
NLOC = N // NC           # 12500 nodes per core
HIB = 98                 # dest hi blocks  (local = hi*128 + lo)
Q = 128                  # source col-block width
NJ = (N + Q - 1) // Q    # 782
GJ = 98                  # J blocks per hw-loop iteration (784 = 98*8)
NJP = ((NJ + GJ - 1) // GJ) * GJ   # 784
NG = NJP // GJ           # 8 loop iterations
JB = 14                  # J-blocks per qb staging DMA (98 = 7*14; <=12
                         # dynamic-broadcast DMAs per For_i body are allowed)
H = 16

_TRACE = bool(int(os.environ.get("KERNEL_TRACE", "0")))
_LAST_TRACE = {}


def _host_prep(x, edge_index, W1, b1, W2, b2):
    x = np.asarray(x, np.float32).reshape(-1)
    ei = np.asarray(edge_index)
    row = ei[0].astype(np.int64)
    col = ei[1].astype(np.int64)

    deg = np.bincount(row, minlength=N).astype(np.float32)

    core = row // NLOC
    J = col // Q
    q = (col % Q).astype(np.float32)
    loc = row % NLOC
    lo = (loc % 128).astype(np.float32)
    hi = (loc // 128).astype(np.float32)

    # per (core, J) bucket sizes -> global uniform S
    cnt = np.bincount(core * NJP + J, minlength=NC * NJP).reshape(NC, NJP)
    S = int(np.max((cnt + 127) // 128))
    C = NJP * S

    order = np.lexsort((J, core))
    J_s = J[order]
    core_s = core[order]
    bucket_id = core_s * NJP + J_s
    starts = np.zeros(NC * NJP + 1, np.int64)
    np.cumsum(np.bincount(bucket_id, minlength=NC * NJP), out=starts[1:])
    k = np.arange(order.size, dtype=np.int64) - starts[bucket_id]

    # slot layout: qd flat index = (J*S + s)*128 + e ; lo/hi at [e, J*S + s]
    chunk = J_s * S + k // 128
    part = k % 128

    q_o = q[order]
    lo_o = lo[order]
    hi_o = hi[order]

    CG = GJ * S
    per_core = []
    for c in range(NC):
        m = core_s == c
        qs = np.zeros(C * 128, np.float32)
        los = np.full(128 * C, 127.0, np.float32)
        his = np.full(128 * C, 97.0, np.float32)
        ch = chunk[m]
        pt = part[m]
        qs[ch * 128 + pt] = q_o[m]
        los[pt * C + ch] = lo_o[m]
        his[pt * C + ch] = hi_o[m]
        # merged lo/hi stream, group-major: [128, NG, {lo,hi}, CG]
        lh = np.empty((128, NG, 2, CG), np.float32)
        lh[:, :, 0, :] = los.reshape(128, NG, CG)
        lh[:, :, 1, :] = his.reshape(128, NG, CG)

        deg_loc = np.zeros(HIB * 128, np.float32)
        x_loc = np.zeros(HIB * 128, np.float32)
        deg_loc[:NLOC] = deg[c * NLOC:(c + 1) * NLOC]
        x_loc[:NLOC] = x[c * NLOC:(c + 1) * NLOC]

        per_core.append(dict(
            qd=qs.astype(np.float16).reshape(1, C * 128),
            lhd=lh.reshape(128, 2 * C),
            deg_loc=deg_loc.reshape(HIB, 128),
            x_loc=x_loc.reshape(HIB, 128),
        ))

    # col-block global tables (node n -> [n%128, n//128]), zero padded
    x_cb = np.zeros(128 * NJP, np.float32)
    deg_cb = np.zeros(128 * NJP, np.float32)
    idx = np.arange(N)
    cbf = (idx % 128) * NJP + idx // 128
    x_cb[cbf] = x
    deg_cb[cbf] = deg
    x_cb = x_cb.reshape(128, NJP)
    deg_cb = deg_cb.reshape(128, NJP)

    params = np.zeros(81, np.float32)
    params[0:16] = np.asarray(W1, np.float32)[0, 0]
    params[16:32] = np.asarray(W1, np.float32)[1, 0]
    params[32:48] = np.asarray(b1, np.float32)
    params[48:64] = np.asarray(W2, np.float32)[0, :, 0]
    params[64:80] = np.asarray(W2, np.float32)[1, :, 0]
    params[80] = np.asarray(b2, np.float32).reshape(-1)[0]
    params = params.reshape(1, 81)

    in_maps = []
    for c in range(NC):
        d = per_core[c]
        in_maps.append({
            "qd": d["qd"], "lhd": d["lhd"],
            "x_cb": x_cb, "deg_cb": deg_cb,
            "x_loc": d["x_loc"], "deg_loc": d["deg_loc"],
            "params": params,
        })
    return S, C, in_maps


def _split_drain_waits(js: bytes) -> bytes:
    """This walrus build rejects >1 sync-wait per instruction; carry excess
    waits on preceding same-engine NoOps (engines dispatch in order)."""
    m = json.loads(js)

    def fix_block(bb):
        insts = bb.get("instructions")
        if not insts:
            return
        out = []
        for inst in insts:
            si = inst.get("sync_info") or {}
            waits = si.get("on_wait") or []
            if len(waits) > 1:
                for kk, w in enumerate(waits[:-1]):
                    carrier = {
                        "opcode": "NoOp",
                        "engine": inst.get("engine", "SP"),
                        "name": f"{inst['name']}_sw{kk}",
                        "ins": [],
                        "outs": [],
                        "sync_info": {"on_update": [], "on_wait": [w]},
                    }
                    if "debug" in inst:
                        carrier["debug"] = inst["debug"]
                    out.append(carrier)
                inst["sync_info"]["on_wait"] = [waits[-1]]
            out.append(inst)
        bb["instructions"] = out

    def walk(o):
        if isinstance(o, dict):
            if isinstance(o.get("instructions"), list):
                fix_block(o)
            for v in o.values():
                walk(v)
        elif isinstance(o, list):
            for v in o:
                walk(v)

    walk(m)
    return json.dumps(m).encode()


def _build_nc(S):
    import concourse.bass as bass
    import concourse.mybir as mybir
    import concourse.tile as tile
    from contextlib import ExitStack

    f32 = mybir.dt.float32
    f16 = mybir.dt.float16
    i32 = mybir.dt.int32
    op = mybir.AluOpType
    C = NJP * S
    CG = GJ * S          # chunk-columns per loop iteration

    nc = bass.Bass()
    qd = nc.declare_dram_parameter("qd", [1, C * 128], f16, isOutput=False)
    lhd = nc.declare_dram_parameter("lhd", [128, 2 * C], f32, isOutput=False)
    x_cb = nc.declare_dram_parameter("x_cb", [128, NJP], f32, isOutput=False)
    deg_cb = nc.declare_dram_parameter("deg_cb", [128, NJP], f32, isOutput=False)
    x_locp = nc.declare_dram_parameter("x_loc", [HIB, 128], f32, isOutput=False)
    deg_locp = nc.declare_dram_parameter("deg_loc", [HIB, 128], f32, isOutput=False)
    paramsp = nc.declare_dram_parameter("params", [1, 81], f32, isOutput=False)
    outp = nc.declare_dram_parameter("outp", [HIB, 128], f32, isOutput=True)

    with ExitStack() as ctx:
        tc = ctx.enter_context(tile.TileContext(nc))
        const = ctx.enter_context(tc.tile_pool(name="const", bufs=1))
        qbp = ctx.enter_context(tc.tile_pool(name="qbp", bufs=2))
        oqp = ctx.enter_context(tc.tile_pool(name="oqp", bufs=3))
        work = ctx.enter_context(tc.tile_pool(name="work", bufs=6))
        lhp = ctx.enter_context(tc.tile_pool(name="lhp", bufs=2))
        msgp = ctx.enter_context(tc.tile_pool(name="msgp", bufs=3))
        nodew = ctx.enter_context(tc.tile_pool(name="nodew", bufs=1))
        psum = ctx.enter_context(tc.tile_pool(name="psum", bufs=1, space="PSUM"))
        psmsg = ctx.enter_context(tc.tile_pool(name="psmsg", bufs=3, space="PSUM"))
        dram = ctx.enter_context(tc.tile_pool(name="dram", bufs=1, space="DRAM"))

        # ---------- constants ----------
        iota_i = const.tile([128, 128], i32)
        nc.gpsimd.iota(iota_i[:], pattern=[[1, 128]], base=0, channel_multiplier=0)
        iotaf = const.tile([128, 128], f16)
        nc.vector.tensor_copy(out=iotaf[:], in_=iota_i[:])
        iotap_i = const.tile([128, 1], i32)
        nc.gpsimd.iota(iotap_i[:], pattern=[[0, 1]], base=0, channel_multiplier=1)
        iotap = const.tile([128, 1], f32)
        nc.vector.tensor_copy(out=iotap[:], in_=iotap_i[:])
        ones_col = const.tile([128, 1], f16)
        nc.gpsimd.memset(ones_col[:], 1.0)
        ZTb = const.tile([128, 128], f16)
        nc.gpsimd.memset(ZTb[:], 0.0)
        P81 = const.tile([128, 81], f32)
        nc.sync.dma_start(out=P81[:], in_=paramsp[0:1, :].to_broadcast([128, 81]))

        # ---------- helpers ----------
        def newton_dinv(dst, deg_tile, F, P=128):
            m = nodew.tile([P, F], f32, name=f"nt_m{F}", tag=f"nt_m{F}")
            r0 = nodew.tile([P, F], f32, name=f"nt_r0{F}", tag=f"nt_r0{F}")
            t = nodew.tile([P, F], f32, name=f"nt_t{F}", tag=f"nt_t{F}")
            nc.vector.tensor_scalar(out=m[:], in0=deg_tile[:], scalar1=1.0,
                                    scalar2=None, op0=op.max)
            nc.scalar.activation(t[:], m[:], mybir.ActivationFunctionType.Sqrt)
            nc.vector.reciprocal(r0[:], t[:])
            nc.vector.tensor_tensor(out=t[:], in0=r0[:], in1=r0[:], op=op.mult)
            nc.vector.tensor_tensor(out=t[:], in0=t[:], in1=m[:], op=op.mult)
            nc.vector.tensor_scalar(out=t[:], in0=t[:], scalar1=-0.5,
                                    scalar2=1.5, op0=op.mult, op1=op.add)
            nc.vector.tensor_tensor(out=t[:], in0=t[:], in1=r0[:], op=op.mult)
            nc.vector.tensor_scalar(out=m[:], in0=deg_tile[:], scalar1=0.0,
                                    scalar2=None, op0=op.not_equal)
            nc.vector.tensor_tensor(out=dst[:], in0=t[:], in1=m[:], op=op.mult)

        # ---------- node tables ----------
        xcb_t = nodew.tile([128, NJP], f32)
        degcb_t = nodew.tile([128, NJP], f32)
        nc.sync.dma_start(out=xcb_t[:], in_=x_cb[:])
        nc.sync.dma_start(out=degcb_t[:], in_=deg_cb[:])
        dinv_cb = nodew.tile([128, NJP], f32)
        newton_dinv(dinv_cb, degcb_t, NJP)
        y_cb = nodew.tile([128, NJP], f32)
        nc.vector.tensor_tensor(out=y_cb[:], in0=dinv_cb[:], in1=xcb_t[:], op=op.mult)

        xl_t = nodew.tile([HIB, 128], f32)
        degl_t = nodew.tile([HIB, 128], f32)
        nc.sync.dma_start(out=xl_t[:], in_=x_locp[:])
        nc.sync.dma_start(out=degl_t[:], in_=deg_locp[:])
        dinv_loc = nodew.tile([HIB, 128], f32)
        newton_dinv(dinv_loc, degl_t, 128, P=HIB)

        # gather tables live in DRAM (dynamic-offset DMA source must be DRAM)
        ycb_d = dram.tile([128, NJP], f32)
        nc.sync.dma_start(out=ycb_d[:], in_=y_cb[:])
        zcb_d = dram.tile([128, NJP], f32)
        z_flat = dram.tile([1, NLOC], f32)
        z_all = dram.tile([1, NJP * 128], f32)

        import concourse.bass as _b

        def emit_pass(table_cb, s_out, pidx):
            G = psum.tile([HIB, 128], f32, tag=f"G{pidx}", name=f"G{pidx}")
            nc.tensor.matmul(out=G[:], lhsT=ZTb[:, :HIB], rhs=ZTb[:],
                             start=True, stop=False, skip_group_check=True)
            with tc.For_i(0, NG) as g:
                lh_t = lhp.tile([128, 2 * CG], f32, name="lh_t", tag="lh_t")
                nc.sync.dma_start(out=lh_t[:], in_=lhd[:, _b.ts(g, 2 * CG)])
                lo_t = lh_t[:, 0:CG]
                hi_t = lh_t[:, CG:2 * CG]
                ycols = lhp.tile([128, GJ], f32, name="ycols", tag="ycols")
                nc.sync.dma_start(out=ycols[:], in_=table_cb[:, _b.ts(g, GJ)])
                ycols16 = lhp.tile([128, GJ], f16, name="ycols16", tag="ycols16")
                nc.vector.tensor_copy(out=ycols16[:], in_=ycols[:])
                # dynamic->static offset staging: one dynamic DRAM->DRAM copy,
                # then static-offset broadcasts (no bounds-check registers)
                qscr = dram.tile([1, CG * 128], f16, name="qscr", tag="qscr",
                                 bufs=2)
                nc.sync.dma_start(out=qscr[0:1, 0:CG * 64],
                                  in_=qd[0:1, _b.ts(2 * g, CG * 64)])
                nc.sync.dma_start(out=qscr[0:1, CG * 64:CG * 128],
                                  in_=qd[0:1, _b.ts(2 * g + 1, CG * 64)])
                for t in range(GJ // JB):
                    qb = qbp.tile([128, JB * S * 128], f16, name="qb", tag="qb")
                    nc.sync.dma_start(
                        out=qb[:],
                        in_=qscr[0:1, t * JB * S * 128:(t + 1) * JB * S * 128]
                        .to_broadcast([128, JB * S * 128]),
                    )
                    for u in range(JB):
                        jj = t * JB + u
                        oqs = oqp.tile([128, S * 128], f16, name="oqs", tag="oqs")
                        nc.vector.tensor_scalar(
                            out=oqs[:], in0=qb[:, u * S * 128:(u + 1) * S * 128],
                            scalar1=iotap[:], scalar2=None, op0=op.is_equal)
                        msg_ps = psmsg.tile([128, S], f32, name="msg_ps", tag="msg_ps")
                        for s in range(S):
                            nc.tensor.matmul(
                                out=msg_ps[:, s:s + 1],
                                lhsT=oqs[:, 128 * s:128 * (s + 1)],
                                rhs=ycols16[:, jj:jj + 1], start=True, stop=True,
                                skip_group_check=True)
                        msg_sb = msgp.tile([128, S], f32, name="msg_sb", tag="msg_sb")
                        nc.scalar.activation(msg_sb[:], msg_ps[:],
                                             mybir.ActivationFunctionType.Copy)
                        # Act materializes hi replicated x98; one wide DVE
                        # tensor_tensor builds all S chunks' Ohm at once
                        hs = hi_t[:, jj * S:(jj + 1) * S]
                        hrep = work.tile([128, S * HIB], f16, tag="hrep",
                                         name="hrep")
                        nc.scalar.activation(
                            hrep[:],
                            _b.AP(hs.tensor, hs.offset,
                                  [list(hs.ap[0]), [1, S], [0, HIB]]),
                            mybir.ActivationFunctionType.Copy)
                        iof98 = iotaf[:, 0:HIB]
                        ohmw = work.tile([128, S * HIB], f16, tag="ohmw",
                                         name="ohmw")
                        nc.vector.tensor_tensor(
                            out=ohmw[:].rearrange("p (s w) -> p s w", w=HIB),
                            in0=hrep[:].rearrange("p (s w) -> p s w", w=HIB),
                            in1=_b.AP(iof98.tensor, iof98.offset,
                                      [list(iof98.ap[0]), [0, S], [1, HIB]]),
                            op=op.is_equal)
                        for s in range(S):
                            cix = jj * S + s
                            ol = work.tile([128, 128], f16, tag="ol", name="ol")
                            nc.vector.tensor_scalar(
                                out=ol[:], in0=iotaf[:],
                                scalar1=lo_t[:, cix:cix + 1],
                                scalar2=msg_sb[:, s:s + 1],
                                op0=op.is_equal, op1=op.mult)
                            nc.tensor.matmul(out=G[:],
                                             lhsT=ohmw[:, s * HIB:(s + 1) * HIB],
                                             rhs=ol[:],
                                             start=False, stop=False,
                                             skip_group_check=True)
            nc.tensor.matmul(out=G[:], lhsT=ZTb[:, :HIB], rhs=ZTb[:],
                             start=False, stop=True, skip_group_check=True)
            nc.vector.tensor_copy(out=s_out[:], in_=G[:])

        # =================== pass 1 ===================
        s1 = nodew.tile([HIB, 128], f32)
        emit_pass(ycb_d, s1, 0)

        # Tx1 = -dinv_loc * s1
        tx1 = nodew.tile([HIB, 128], f32)
        nc.vector.scalar_tensor_tensor(out=tx1[:], in0=s1[:], scalar=-1.0,
                                       in1=dinv_loc[:], op0=op.mult, op1=op.mult)
        g2 = [nodew.tile([HIB, 128], f32, name=f"g2_{i}", tag=f"g2{i}") for i in range(2)]
        p2 = [nodew.tile([HIB, 128], f32, name=f"p2_{i}", tag=f"p2{i}") for i in range(2)]
        nc.gpsimd.memset(g2[0][:], 0.0)
        nc.gpsimd.memset(p2[0][:], 0.0)
        tv = nodew.tile([HIB, 128], f32)
        hch = nodew.tile([HIB, 128], f32)
        for ch in range(H):
            u_c = P81[0:HIB, ch:ch + 1]
            v_c = P81[0:HIB, 16 + ch:17 + ch]
            b1_c = P81[0:HIB, 32 + ch:33 + ch]
            w2a_c = P81[0:HIB, 48 + ch:49 + ch]
            w2b_c = P81[0:HIB, 64 + ch:65 + ch]
            nc.vector.tensor_scalar(out=tv[:], in0=tx1[:], scalar1=v_c,
                                    scalar2=None, op0=op.mult)
            nc.vector.scalar_tensor_tensor(out=hch[:], in0=xl_t[:], scalar=u_c,
                                           in1=tv[:], op0=op.mult, op1=op.add)
            nc.vector.tensor_scalar(out=hch[:], in0=hch[:], scalar1=b1_c,
                                    scalar2=0.0, op0=op.add, op1=op.max)
            a, b = ch % 2, 1 - ch % 2
            nc.vector.scalar_tensor_tensor(out=g2[b][:], in0=hch[:], scalar=w2b_c,
                                           in1=g2[a][:], op0=op.mult, op1=op.add)
            nc.vector.scalar_tensor_tensor(out=p2[b][:], in0=hch[:], scalar=w2a_c,
                                           in1=p2[a][:], op0=op.mult, op1=op.add)
        g2f = g2[H % 2]
        p2f = p2[H % 2]

        # z = dinv_loc * g2  -> z_flat -> allgather -> z_all -> z_cb
        zl = nodew.tile([HIB, 128], f32)
        nc.vector.tensor_tensor(out=zl[:], in0=dinv_loc[:], in1=g2f[:], op=op.mult)
        nc.sync.dma_start(
            out=z_flat[0:1, 0:(HIB - 1) * 128].rearrange("o (h l) -> (o h) l", l=128),
            in_=zl[0:HIB - 1, :],
        )
        nc.sync.dma_start(
            out=z_flat[0:1, (HIB - 1) * 128:NLOC],
            in_=zl[HIB - 1:HIB, 0:NLOC - (HIB - 1) * 128],
        )
        zrow = const.tile([1, 512], f32)
        nc.gpsimd.memset(zrow[:], 0.0)
        nc.sync.dma_start(out=z_all[0:1, N:NJP * 128], in_=zrow[0:1, 0:NJP * 128 - N])
        nc.gpsimd.collective_compute(
            "AllGather", op.bypass,
            replica_groups=[list(range(NC))],
            ins=[z_flat[0:1, :]],
            outs=[z_all[0:1, 0:N]],
        )
        # zcb_d[p, j] = z_all[j*128 + p]  (split: flat side <= 65535 elems/DMA)
        NJH = NJP // 2
        for h in range(2):
            nc.sync.dma_start(
                out=zcb_d[:, h * NJH:(h + 1) * NJH],
                in_=z_all[0:1, h * NJH * 128:(h + 1) * NJH * 128]
                .rearrange("o (j p) -> (o p) j", p=128),
            )

        # =================== pass 2 ===================
        s2 = nodew.tile([HIB, 128], f32)
        emit_pass(zcb_d, s2, 1)

        o1 = nodew.tile([HIB, 128], f32)
        nc.vector.scalar_tensor_tensor(out=o1[:], in0=s2[:], scalar=-1.0,
                                       in1=dinv_loc[:], op0=op.mult, op1=op.mult)
        nc.vector.tensor_tensor(out=o1[:], in0=o1[:], in1=p2f[:], op=op.add)
        nc.vector.tensor_scalar(out=o1[:], in0=o1[:], scalar1=P81[0:HIB, 80:81],
                                scalar2=None, op0=op.add)
        nc.sync.dma_start(out=outp[:], in_=o1[:])

    # patch: split multi-wait Drains for this walrus build
    orig = type(nc).to_json_bytes
    if not getattr(type(nc), "_drain_patched", False):
        def patched(self):
            return _split_drain_waits(orig(self))
        type(nc).to_json_bytes = patched
        type(nc)._drain_patched = True
    return nc


def _install_ntff_hook():
    """Recreate the missing antenv.axon_hooks shim so trace=True works."""
    import sys
    import types
    try:
        import antenv.axon_hooks  # noqa: F401
        return True
    except ImportError:
        pass
    try:
        from trn_agent_boot.trn_boot import _ntff_profile_via_ctypes
        hook = _ntff_profile_via_ctypes("/opt/axon/libaxon_pjrt.so")
        if hook is None:
            return False
        mod = types.ModuleType("antenv.axon_hooks")
        mod._hook = hook
        mod.get_axon_ntff_profile_hook = lambda: mod._hook
        mod.set_axon_ntff_profile_hook = lambda h: setattr(mod, "_hook", h)
        import antenv
        antenv.axon_hooks = mod
        sys.modules["antenv.axon_hooks"] = mod
        return True
    except Exception:
        return False


def kernel(x, edge_index, W1, b1, W2, b2):
    from concourse.bass_utils import run_bass_kernel_spmd

    S, C, in_maps = _host_prep(x, edge_index, W1, b1, W2, b2)
    nc = _build_nc(S)
    trace = _TRACE and _install_ntff_hook()
    res = run_bass_kernel_spmd(nc, in_maps, list(range(NC)), trace=trace)
    global _LAST_TRACE
    _LAST_TRACE = {
        "exec_time_ns": res.exec_time_ns,
        "profile_json": getattr(res, "profile_json", None),
    }
    out = np.concatenate(
        [res.results[c]["outp"].reshape(-1)[:NLOC] for c in range(NC)]
    ).astype(np.float32)
    return out.reshape(N, 1)
